# revision 23
# baseline (speedup 1.0000x reference)
"""CAM+SE module kernel for Trainium2, data-parallel over batch across 8 cores.

Reference computation (per sample):
    q = x.reshape(C, HW)
    energy = q @ q.T                      # C x C
    att = softmax(max(energy) - energy)   # row-wise; == exp(mn_c - e) / Z_c
    ch_out = att @ q
    se = sigmoid(relu(mean_hw(x) @ W1 + b1) @ W2 + b2)
    out = gamma * (ch_out * se[:, None]) + x

v3 design:
  - x is loaded via GPSIMD (SWDGE) casting DMAs directly into a bf16 copy
    (transpose source + residual) and an fp8e4 copy in DoubleRow pair
    layout (MM2 moving operand).  No f32 x in SBUF: 12.6MB input instead
    of 16MB, and zero on-chip cast traffic.
  - Both big matmuls run fp8e4 DoubleRow (2 k-tiles per instruction) with
    f32 PSUM accumulation.  Energy is computed in full; softmax is one
    row-min + one exp per 128-row tile.
  - PE transposes are bf16; the PSUM->SBUF evacuation casts to fp8 and
    alternates between ACT and DVE per n-tile so it never paces MM1.
  - SE global-average-pool row sums ride the PE: a ones-vector DoubleRow
    matmul against the same stationary qT blocks MM1 loads (1/HW folded
    into W1).
  - The two samples are software-pipelined: sample 1's transpose/MM1 phase
    is emitted interleaved with sample 0's MM2/store phase so every engine
    sees work from both samples back to back.
  - The residual fused multiply-add (stt) alternates DVE/Pool per chunk;
    output is stored bf16 (host upcasts), halving write traffic.  In the
    gamma=0 regime out == bf16(x) exactly up to bf16 rounding (~1e-3 rel).
"""

import numpy as np

B, C, H, W = 16, 512, 64, 64
HW = H * W
NCORES = 8
BS = B // NCORES          # samples per core
CT = C // 128             # 4 c-tiles
NT = HW // 32 // 128 * 8  # 32 n-tiles
NT = HW // 128            # 32 n-tiles
NP = NT // 2              # 16 n-tile pairs (DoubleRow)
R = C // 8                # 64

_BUILT = None
LAST_RESULTS = None
TRACE = False
CFG = {
    "bf16_chunks": 2,
    "fp8_chunks": 1,
    "qt_bufs": 17,      # all 16 qT pairs retained for the two-pass MM1
    "st_bufs": 4,
    "pc_bufs": 2,
    "tp_bufs": 2,
    # evac engine rotation per sample: during s0's MM1 DVE is idle, share;
    # during s1's (interleaved with s0's stt on DVE) keep evacs on ACT
    "evac_engines": (["scalar", "vector"], ["scalar"]),
    # residual bounce-via-ACT fraction per sample: s0's MM2 overlaps s1's
    # ACT-heavy MM1 (no bounce); s1's MM2 has ACT idle (bounce half)
    "stt_bounce_mod": (0, 2),
    "out_eng": "sync",
    "dma_ring": 49152,
}


def _build():
    global _BUILT
    if _BUILT is not None:
        return _BUILT

    import concourse.bacc as bacc
    import concourse.mybir as mybir
    import concourse.tile as tile
    from concourse.masks import make_identity

    f32 = mybir.dt.float32
    bf16 = mybir.dt.bfloat16
    fp8 = mybir.dt.float8e4
    ALU = mybir.AluOpType
    ACT = mybir.ActivationFunctionType
    DR = mybir.MatmulPerfMode.DoubleRow

    nc = bacc.Bacc(
        "TRN2",
        target_bir_lowering=False,
        debug=False,
        enable_asserts=False,
        num_devices=NCORES,
        dynamic_dma_scratch_size=CFG["dma_ring"],
    )

    x_d = nc.dram_tensor("x", (BS, C, HW), f32, kind="ExternalInput").ap()
    w1_d = nc.dram_tensor("w1", (C, R), f32, kind="ExternalInput").ap()
    b1_d = nc.dram_tensor("b1", (R, 1), f32, kind="ExternalInput").ap()
    w2_d = nc.dram_tensor("w2", (R, C), f32, kind="ExternalInput").ap()
    b2_d = nc.dram_tensor("b2", (C, 1), f32, kind="ExternalInput").ap()
    g_d = nc.dram_tensor("gam", (1, 1), f32, kind="ExternalInput").ap()
    out_d = nc.dram_tensor("out", (BS, C, HW), bf16, kind="ExternalOutput").ap()

    with tile.TileContext(nc) as tc:
        with (
            tc.tile_pool(name="qbpool", bufs=2) as qbpool,
            tc.tile_pool(name="q8pool", bufs=2) as q8pool,
            tc.tile_pool(name="qtpool", bufs=CFG["qt_bufs"]) as qtpool,
            tc.tile_pool(name="ppool", bufs=2) as ppool,
            tc.tile_pool(name="ptpool", bufs=2) as ptpool,
            tc.tile_pool(name="stpool", bufs=CFG["st_bufs"]) as stpool,
            tc.tile_pool(name="stat", bufs=2) as stat,
            tc.tile_pool(name="constp", bufs=1) as constp,
            tc.tile_pool(name="epool", bufs=1, space="PSUM") as epool,
            tc.tile_pool(name="tppool", bufs=CFG["tp_bufs"], space="PSUM") as tppool,
            tc.tile_pool(name="pcpool", bufs=CFG["pc_bufs"], space="PSUM") as pcpool,
            tc.tile_pool(name="scpool", bufs=1, space="PSUM") as scpool,
        ):
            # ---- constants ----
            ident = constp.tile([128, 128], f32, name="ident")
            make_identity(nc, ident)
            ident_b = constp.tile([128, 128], bf16, name="identb")
            nc.vector.tensor_copy(ident_b, ident)
            ones8 = constp.tile([128, 2, 1], fp8, name="ones8")
            nc.vector.memset(ones8, 1.0)

            def emit_params():
                w1s = []
                for k in range(CT):
                    w1raw = constp.tile([128, R], f32, name=f"w1raw{k}")
                    nc.scalar.dma_start(w1raw, w1_d[128 * k:128 * (k + 1), :])
                    w1k = constp.tile([128, R], f32, name=f"w1s{k}")
                    # fold the 1/HW of the global average pool into W1
                    nc.vector.tensor_scalar_mul(w1k, w1raw, 1.0 / HW)
                    w1s.append(w1k)

                w2_sb = constp.tile([R, C], f32, name="w2sb")
                nc.scalar.dma_start(w2_sb, w2_d)
                b1_sb = constp.tile([R, 1], f32, name="b1sb")
                nc.scalar.dma_start(b1_sb, b1_d)
                negb2 = []
                for m in range(CT):
                    b2raw = constp.tile([128, 1], f32, name=f"b2raw{m}")
                    nc.scalar.dma_start(b2raw, b2_d[128 * m:128 * (m + 1), :])
                    nb2 = constp.tile([128, 1], f32, name=f"negb2{m}")
                    nc.vector.tensor_scalar_mul(nb2, b2raw, -1.0)
                    negb2.append(nb2)

                g_sb = constp.tile([1, 1], f32, name="gsb")
                nc.scalar.dma_start(g_sb, g_d)
                g128 = constp.tile([128, 1], f32, name="g128")
                nc.gpsimd.partition_broadcast(g128, g_sb[0:1, :])
                return w1s, w2_sb, b1_sb, negb2, g128

            def emit_load(s):
                """SWDGE casting DMAs: f32 HBM -> bf16 tiles + fp8 pair tiles.

                bf16 first, chunked, so the transpose pipeline starts as
                soon as the first n-chunk of all four c-tiles has landed;
                fp8 after (only needed by MM2, much later).
                """
                qb = []
                nchb = CFG["bf16_chunks"]
                bsz = HW // nchb
                for i in range(CT):
                    qb_i = qbpool.tile([128, HW], bf16, name=f"qb{i}", tag=f"qb{i}")
                    qb.append(qb_i)
                for cc in range(nchb):
                    csl = slice(bsz * cc, bsz * (cc + 1))
                    for i in range(CT):
                        nc.gpsimd.dma_start(
                            qb[i][:, csl], x_d[s, 128 * i:128 * (i + 1), csl]
                        )
                q8p = []
                for k in range(2):
                    t8 = q8pool.tile(
                        [128, 2, HW], fp8, name=f"q8_{k}", tag=f"q8{k}"
                    )
                    q8p.append(t8)
                nch = CFG["fp8_chunks"]
                csz = HW // nch
                for cc in range(nch):
                    csl = slice(csz * cc, csz * (cc + 1))
                    for i in range(CT):
                        nc.gpsimd.dma_start(
                            q8p[i // 2][:, i % 2, csl],
                            x_d[s, 128 * i:128 * (i + 1), csl],
                        )
                return q8p, qb

            def copy_eng(eng_name, dst, src):
                if eng_name == "scalar":
                    nc.scalar.copy(dst, src)
                elif eng_name == "vector":
                    nc.vector.tensor_copy(dst, src)
                else:
                    nc.gpsimd.tensor_copy(dst, src)

            def softmax_pair(s, ctx, i0):
                """Row-min + exp + 1/Z for row-tiles i0, i0+1 (frees both
                energy banks for the next pass / the PT transposes)."""
                mns, Zs = {}, {}
                for i in (i0, i0 + 1):
                    mn = stat.tile([128, 1], f32, name=f"mn{i}", tag=f"mn{i}")
                    nc.vector.tensor_reduce(
                        mn, ctx["e"][i], axis=mybir.AxisListType.X, op=ALU.min
                    )
                    mns[i] = mn
                for i in (i0, i0 + 1):
                    P_i = ppool.tile([128, 512], bf16, name=f"P{i}", tag=f"P{i}")
                    Zt = stat.tile([128, 1], f32, name=f"Z{i}", tag=f"Z{i}")
                    nc.scalar.activation(
                        P_i, ctx["e"][i], ACT.Exp, bias=mns[i], scale=-1.0,
                        accum_out=Zt,
                    )
                    ctx["P"][i] = P_i
                    Zs[i] = Zt
                for i in (i0, i0 + 1):
                    rz = stat.tile([128, 1], f32, name=f"rz{i}", tag=f"rz{i}")
                    nc.vector.reciprocal(rz, Zs[i])
                    ctx["rZ"][i] = rz

            def mm1_steps(s, q8p, qb):
                """Pass A (per n-tile): 4 transposes -> tp, evac -> qT pair
                slot; on odd n-tiles the pair's m=0,1 MM1 + SE-ones matmuls.
                Pass B (per pair): m=2,3 matmuls re-reading the retained qT
                tiles into the recycled energy banks."""
                ctx = {
                    "e": [None] * CT,
                    "scol": scpool.tile([128, CT], f32, name=f"scol{s}", tag=f"sc{s}"),
                    "qT": {},
                    "P": [None] * CT,
                    "rZ": [None] * CT,
                }
                ctx["e"][0] = epool.tile([128, 512], f32, name=f"e0_{s}", tag="eA")
                ctx["e"][1] = epool.tile([128, 512], f32, name=f"e1_{s}", tag="eB")
                evac = CFG["evac_engines"][s % len(CFG["evac_engines"])]

                def pair_mms(p, m0, m1, first, last):
                    cur = ctx["qT"][p]
                    for m in (m0, m1):
                        lhsT = cur[:, :, 128 * m:128 * (m + 1)]
                        nc.tensor.matmul(
                            ctx["e"][m][:, 0:256], lhsT, cur[:, :, 0:256],
                            start=first, stop=last, perf_mode=DR,
                        )
                        nc.tensor.matmul(
                            ctx["e"][m][:, 256:512], lhsT, cur[:, :, 256:512],
                            start=first, stop=last, perf_mode=DR,
                        )
                        nc.tensor.matmul(
                            ctx["scol"][:, m:m + 1], lhsT, ones8,
                            start=first, stop=last, perf_mode=DR,
                        )

                def make_stepA(nt):
                    def trans():
                        p, j = divmod(nt, 2)
                        if j == 0:
                            ctx["tp"] = tppool.tile(
                                [128, 2, 512], bf16, name="tp", tag="tp"
                            )
                        tp = ctx["tp"]
                        for i in range(CT):
                            nc.tensor.transpose(
                                tp[:, j, 128 * i:128 * (i + 1)],
                                qb[i][:, 128 * nt:128 * (nt + 1)],
                                ident_b,
                            )
                        if j == 1:
                            qT = qtpool.tile(
                                [128, 2, 512], fp8, name="qT", tag="qT"
                            )
                            ctx["qT"][p] = qT
                            copy_eng(evac[p % len(evac)], qT, tp)

                    def mms():
                        if nt % 2 == 0:
                            return
                        p = nt // 2
                        pair_mms(p, 0, 1, p == 0, p == NP - 1)

                    return trans, mms

                def make_stepB(p):
                    def stepB():
                        if p == 0:
                            ctx["e"][2] = epool.tile(
                                [128, 512], f32, name=f"e2_{s}", tag="eA"
                            )
                            ctx["e"][3] = epool.tile(
                                [128, 512], f32, name=f"e3_{s}", tag="eB"
                            )
                        pair_mms(p, 2, 3, p == 0, p == NP - 1)
                        if p == NP - 1:
                            del ctx["qT"]

                    return stepB

                stepsA = [make_stepA(nt) for nt in range(NT)]
                stepsB = [make_stepB(p) for p in range(NP)]
                return ctx, stepsA, stepsB

            def mm1_flat(ctx_s, stepsA, s, lookahead=2):
                """Pass-A emission order with transpose lookahead, as a
                flat list of closures."""
                n = len(stepsA)
                work = [stepsA[k][0] for k in range(lookahead)]
                for nt in range(n):
                    if nt + lookahead < n:
                        work.append(stepsA[nt + lookahead][0])
                    work.append(stepsA[nt][1])
                return work

            def se_pt(s, ctx, params):
                w1s, w2_sb, b1_sb, negb2, g128 = params
                # SE MLP (tiny, f32)
                scol_sb = stat.tile([128, CT], f32, name="scol_sb", tag="scs")
                nc.vector.tensor_copy(scol_sb, ctx["scol"])
                hp = scpool.tile([64, 1], f32, name="hp", tag=f"sc{s}")
                for k in range(CT):
                    nc.tensor.matmul(
                        hp, w1s[k], scol_sb[:, k:k + 1],
                        start=(k == 0), stop=(k == CT - 1),
                    )
                h = stat.tile([64, 1], f32, name="h", tag="h")
                nc.scalar.activation(h, hp, ACT.Relu, bias=b1_sb, scale=1.0)

                alph = []
                for m in range(CT):
                    sp = scpool.tile([128, 1], f32, name=f"sp{m}", tag=f"sc{s}")
                    nc.tensor.matmul(sp, w2_sb[:, 128 * m:128 * (m + 1)], h)
                    u = stat.tile([128, 1], f32, name=f"u{m}", tag=f"u{m}")
                    nc.scalar.activation(u, sp, ACT.Exp, bias=negb2[m], scale=-1.0)
                    t1 = stat.tile([128, 1], f32, name=f"t1{m}", tag=f"t1{m}")
                    nc.vector.tensor_scalar_add(t1, u, 1.0)
                    sig = stat.tile([128, 1], f32, name=f"sig{m}", tag=f"sig{m}")
                    nc.vector.reciprocal(sig, t1)
                    a1 = stat.tile([128, 1], f32, name=f"a1{m}", tag=f"a1{m}")
                    nc.vector.tensor_mul(a1, sig, rZ_of(ctx, m))
                    a2 = stat.tile([128, 1], f32, name=f"a2{m}", tag=f"a2{m}")
                    nc.vector.tensor_mul(a2, a1, g128)
                    alph.append(a2)

                # P -> PT pair transposes (reuse energy PSUM banks)
                Ps = ctx["P"]
                ptps = [
                    epool.tile(
                        [128, 2, 512], bf16, name=f"ptp{k}",
                        tag=("eA" if k == 0 else "eB"),
                    )
                    for k in range(2)
                ]
                for i in range(CT):
                    for kk in range(CT):
                        nc.tensor.transpose(
                            ptps[kk // 2][:, kk % 2, 128 * i:128 * (i + 1)],
                            Ps[i][:, 128 * kk:128 * (kk + 1)],
                            ident_b,
                        )
                PTs = []
                for k in range(2):
                    PT_k = ptpool.tile(
                        [128, 2, 512], fp8, name=f"PT{k}", tag=f"PT{k}"
                    )
                    nc.scalar.copy(PT_k, ptps[k])
                    PTs.append(PT_k)
                return alph, PTs

            def rz_of(ctx, m):
                return ctx["rZ"][m]

            rZ_of = rz_of

            def mm2_steps(s, q8p, qb, alph, PTs):
                """8 closures (m, chg): 4 512-chunks of MM2 + residual + 1
                store.  Most chunks: fused DVE stt straight from PSUM
                (658ns).  Every stt_bounce_mod-th chunk: ACT evacuates
                alpha*pc to bf16 SBUF (612ns, scale fused) and DVE does an
                all-SBUF bf16 add at the 2x rate (327ns) — shifts load from
                DVE to ACT."""
                out_eng = getattr(nc, CFG["out_eng"])
                bmods = CFG["stt_bounce_mod"]
                bmod = bmods[s % len(bmods)]

                def make_step(m, chg):
                    def step():
                        lhs0 = PTs[0][:, :, 128 * m:128 * (m + 1)]
                        lhs1 = PTs[1][:, :, 128 * m:128 * (m + 1)]
                        st = stpool.tile([128, 2048], bf16, name="st", tag="st")
                        for c2 in range(4):
                            ch = 4 * chg + c2
                            nsl = slice(512 * ch, 512 * (ch + 1))
                            pc = pcpool.tile([128, 512], f32, name="pc", tag="pc")
                            h0 = slice(512 * ch, 512 * ch + 256)
                            h1 = slice(512 * ch + 256, 512 * (ch + 1))
                            # k order 0,1,1,0 shares the middle ldweights
                            nc.tensor.matmul(
                                pc[:, 0:256], lhs0, q8p[0][:, :, h0],
                                start=True, stop=False, perf_mode=DR,
                            )
                            nc.tensor.matmul(
                                pc[:, 0:256], lhs1, q8p[1][:, :, h0],
                                start=False, stop=True, perf_mode=DR,
                            )
                            nc.tensor.matmul(
                                pc[:, 256:512], lhs1, q8p[1][:, :, h1],
                                start=True, stop=False, perf_mode=DR,
                            )
                            nc.tensor.matmul(
                                pc[:, 256:512], lhs0, q8p[0][:, :, h1],
                                start=False, stop=True, perf_mode=DR,
                            )
                            ssl = slice(512 * c2, 512 * (c2 + 1))
                            if bmod and ch % bmod == bmod - 1:
                                pcs = stpool.tile(
                                    [128, 512], bf16, name="pcs", tag="pcs",
                                    bufs=3,
                                )
                                nc.scalar.activation(
                                    pcs, pc, ACT.Copy, scale=alph[m]
                                )
                                nc.vector.tensor_tensor(
                                    st[:, ssl], pcs, qb[m][:, nsl], op=ALU.add
                                )
                            else:
                                nc.vector.scalar_tensor_tensor(
                                    st[:, ssl], pc, alph[m], qb[m][:, nsl],
                                    op0=ALU.mult, op1=ALU.add,
                                )
                        out_eng.dma_start(
                            out_d[s, 128 * m:128 * (m + 1),
                                  2048 * chg:2048 * (chg + 1)],
                            st,
                        )

                    return step

                return [make_step(m, chg) for m in range(CT) for chg in range(2)]

            # ---- schedule ----
            loads = {0: emit_load(0)}
            params = emit_params()
            if BS > 1:
                loads[1] = emit_load(1)

            q8p0, qb0 = loads[0]
            ctx0, stepsA0, stepsB0 = mm1_steps(0, q8p0, qb0)
            for w in mm1_flat(ctx0, stepsA0, 0):
                w()

            # s0's remaining work as a step list so s1's MM1 pipeline can
            # interleave with ALL of it (softmax, pass B, SE/PT, MM2)
            s0_rest = [lambda: softmax_pair(0, ctx0, 0)]
            s0_rest.extend(stepsB0)
            s0_rest.append(lambda: softmax_pair(0, ctx0, 2))
            m2_0_holder = {}

            def s0_sept():
                alph0, PTs0 = se_pt(0, ctx0, params)
                m2_0_holder["m2"] = mm2_steps(0, q8p0, qb0, alph0, PTs0)

            s0_rest.append(s0_sept)

            def s0_mm2_step(i):
                def f():
                    m2_0_holder["m2"][i]()
                return f

            s0_rest.extend(s0_mm2_step(i) for i in range(2 * CT))

            if BS > 1:
                q8p1, qb1 = loads[1]
                ctx1, stepsA1, stepsB1 = mm1_steps(1, q8p1, qb1)

                s1_work = mm1_flat(ctx1, stepsA1, 1)
                s1_work.append(lambda: softmax_pair(1, ctx1, 0))
                s1_work.extend(stepsB1)
                s1_work.append(lambda: softmax_pair(1, ctx1, 2))

                # round-robin the two lists: s1's MM1 pipeline threads
                # through s0's softmax/passB/SE/MM2 phase
                per = (len(s1_work) + len(s0_rest) - 1) // len(s0_rest)
                k = 0
                for f0 in s0_rest:
                    f0()
                    for _ in range(per):
                        if k < len(s1_work):
                            s1_work[k]()
                            k += 1
                while k < len(s1_work):
                    s1_work[k]()
                    k += 1

                alph1, PTs1 = se_pt(1, ctx1, params)
                for mstep in mm2_steps(1, q8p1, qb1, alph1, PTs1):
                    mstep()
            else:
                for f0 in s0_rest:
                    f0()

    nc.compile()
    _BUILT = nc
    return nc


def kernel(**inputs):
    global LAST_RESULTS
    from concourse.bass_utils import run_bass_kernel_spmd

    x = np.ascontiguousarray(np.asarray(inputs["x"], dtype=np.float32))
    gamma = np.asarray(inputs["gamma"], dtype=np.float32)
    W1 = np.ascontiguousarray(np.asarray(inputs["W1"], dtype=np.float32))
    b1 = np.asarray(inputs["b1"], dtype=np.float32)
    W2 = np.ascontiguousarray(np.asarray(inputs["W2"], dtype=np.float32))
    b2 = np.asarray(inputs["b2"], dtype=np.float32)

    nc = _build()

    xr = x.reshape(B, C, HW)
    b1c = np.ascontiguousarray(b1.reshape(R, 1))
    b2c = np.ascontiguousarray(b2.reshape(C, 1))
    gc = np.ascontiguousarray(gamma.reshape(1, 1))

    in_maps = []
    for c in range(NCORES):
        shard = np.ascontiguousarray(xr[BS * c: BS * (c + 1)])
        in_maps.append(
            {"x": shard, "w1": W1, "b1": b1c, "w2": W2, "b2": b2c, "gam": gc}
        )

    res = run_bass_kernel_spmd(
        nc, in_maps, core_ids=list(range(NCORES)), trace=TRACE
    )
    LAST_RESULTS = res

    out = np.concatenate(
        [np.asarray(r["out"]).astype(np.float32) for r in res.results], axis=0
    )
    return out.reshape(B, C, H, W)


# revision 26
# speedup vs baseline: 1.0124x; 1.0124x over previous
"""CAM+SE module kernel for Trainium2, data-parallel over batch across 8 cores.

Reference computation (per sample):
    q = x.reshape(C, HW)
    energy = q @ q.T                      # C x C
    att = softmax(max(energy) - energy)   # row-wise; == exp(mn_c - e) / Z_c
    ch_out = att @ q
    se = sigmoid(relu(mean_hw(x) @ W1 + b1) @ W2 + b2)
    out = gamma * (ch_out * se[:, None]) + x

v3 design:
  - x is loaded via GPSIMD (SWDGE) casting DMAs directly into a bf16 copy
    (transpose source + residual) and an fp8e4 copy in DoubleRow pair
    layout (MM2 moving operand).  No f32 x in SBUF: 12.6MB input instead
    of 16MB, and zero on-chip cast traffic.
  - Both big matmuls run fp8e4 DoubleRow (2 k-tiles per instruction) with
    f32 PSUM accumulation.  Energy is computed in full; softmax is one
    row-min + one exp per 128-row tile.
  - PE transposes are bf16; the PSUM->SBUF evacuation casts to fp8 and
    alternates between ACT and DVE per n-tile so it never paces MM1.
  - SE global-average-pool row sums ride the PE: a ones-vector DoubleRow
    matmul against the same stationary qT blocks MM1 loads (1/HW folded
    into W1).
  - The two samples are software-pipelined: sample 1's transpose/MM1 phase
    is emitted interleaved with sample 0's MM2/store phase so every engine
    sees work from both samples back to back.
  - The residual fused multiply-add (stt) alternates DVE/Pool per chunk;
    output is stored bf16 (host upcasts), halving write traffic.  In the
    gamma=0 regime out == bf16(x) exactly up to bf16 rounding (~1e-3 rel).
"""

import numpy as np

B, C, H, W = 16, 512, 64, 64
HW = H * W
NCORES = 8
BS = B // NCORES          # samples per core
CT = C // 128             # 4 c-tiles
NT = HW // 32 // 128 * 8  # 32 n-tiles
NT = HW // 128            # 32 n-tiles
NP = NT // 2              # 16 n-tile pairs (DoubleRow)
R = C // 8                # 64

_BUILT = None
LAST_RESULTS = None
TRACE = False
CFG = {
    "bf16_chunks": (4, 2),  # per sample: fine chunks early for fast start
    "fp8_chunks": 1,
    "qt_bufs": 17,      # all 16 qT pairs retained for the two-pass MM1
    "st_bufs": 4,
    "pc_bufs": 3,
    "tp_bufs": 2,
    # evac engine per (sample, half): s0 runs both halves in parallel on
    # ACT+DVE (DVE idle then); s1's evacs stay on ACT (DVE runs s0's stt)
    "evac_engines": (("scalar", "vector"), ("scalar", "scalar")),
    # residual bounce-via-ACT fraction per sample: s0's MM2 overlaps s1's
    # ACT-heavy MM1 (no bounce); s1's MM2 has ACT idle (bounce half)
    "stt_bounce_mod": (0, 2),
    "out_eng": "sync",
    "dma_ring": 49152,
}


def _build():
    global _BUILT
    if _BUILT is not None:
        return _BUILT

    import concourse.bacc as bacc
    import concourse.mybir as mybir
    import concourse.tile as tile
    from concourse.masks import make_identity

    f32 = mybir.dt.float32
    bf16 = mybir.dt.bfloat16
    fp8 = mybir.dt.float8e4
    ALU = mybir.AluOpType
    ACT = mybir.ActivationFunctionType
    DR = mybir.MatmulPerfMode.DoubleRow

    nc = bacc.Bacc(
        "TRN2",
        target_bir_lowering=False,
        debug=False,
        enable_asserts=False,
        num_devices=NCORES,
        dynamic_dma_scratch_size=CFG["dma_ring"],
    )

    x_d = nc.dram_tensor("x", (BS, C, HW), f32, kind="ExternalInput").ap()
    w1_d = nc.dram_tensor("w1", (C, R), f32, kind="ExternalInput").ap()
    b1_d = nc.dram_tensor("b1", (R, 1), f32, kind="ExternalInput").ap()
    w2_d = nc.dram_tensor("w2", (R, C), f32, kind="ExternalInput").ap()
    b2_d = nc.dram_tensor("b2", (C, 1), f32, kind="ExternalInput").ap()
    g_d = nc.dram_tensor("gam", (1, 1), f32, kind="ExternalInput").ap()
    out_d = nc.dram_tensor("out", (BS, C, HW), bf16, kind="ExternalOutput").ap()

    with tile.TileContext(nc) as tc:
        with (
            tc.tile_pool(name="qbpool", bufs=2) as qbpool,
            tc.tile_pool(name="q8pool", bufs=2) as q8pool,
            tc.tile_pool(name="qtpool", bufs=CFG["qt_bufs"]) as qtpool,
            tc.tile_pool(name="ppool", bufs=2) as ppool,
            tc.tile_pool(name="ptpool", bufs=2) as ptpool,
            tc.tile_pool(name="stpool", bufs=CFG["st_bufs"]) as stpool,
            tc.tile_pool(name="stat", bufs=2) as stat,
            tc.tile_pool(name="constp", bufs=1) as constp,
            tc.tile_pool(name="epool", bufs=1, space="PSUM") as epool,
            tc.tile_pool(name="tppool", bufs=CFG["tp_bufs"], space="PSUM") as tppool,
            tc.tile_pool(name="pcpool", bufs=CFG["pc_bufs"], space="PSUM") as pcpool,
            tc.tile_pool(name="scpool", bufs=1, space="PSUM") as scpool,
        ):
            # ---- constants ----
            ident = constp.tile([128, 128], f32, name="ident")
            make_identity(nc, ident)
            ident_b = constp.tile([128, 128], bf16, name="identb")
            nc.vector.tensor_copy(ident_b, ident)
            ones8 = constp.tile([128, 2, 1], fp8, name="ones8")
            nc.vector.memset(ones8, 1.0)

            def emit_params():
                w1s = []
                for k in range(CT):
                    w1raw = constp.tile([128, R], f32, name=f"w1raw{k}")
                    nc.scalar.dma_start(w1raw, w1_d[128 * k:128 * (k + 1), :])
                    w1k = constp.tile([128, R], f32, name=f"w1s{k}")
                    # fold the 1/HW of the global average pool into W1
                    nc.vector.tensor_scalar_mul(w1k, w1raw, 1.0 / HW)
                    w1s.append(w1k)

                w2_sb = constp.tile([R, C], f32, name="w2sb")
                nc.scalar.dma_start(w2_sb, w2_d)
                b1_sb = constp.tile([R, 1], f32, name="b1sb")
                nc.scalar.dma_start(b1_sb, b1_d)
                negb2 = []
                for m in range(CT):
                    b2raw = constp.tile([128, 1], f32, name=f"b2raw{m}")
                    nc.scalar.dma_start(b2raw, b2_d[128 * m:128 * (m + 1), :])
                    nb2 = constp.tile([128, 1], f32, name=f"negb2{m}")
                    nc.vector.tensor_scalar_mul(nb2, b2raw, -1.0)
                    negb2.append(nb2)

                g_sb = constp.tile([1, 1], f32, name="gsb")
                nc.scalar.dma_start(g_sb, g_d)
                g128 = constp.tile([128, 1], f32, name="g128")
                nc.gpsimd.partition_broadcast(g128, g_sb[0:1, :])
                return w1s, w2_sb, b1_sb, negb2, g128

            def emit_load(s):
                """SWDGE casting DMAs: f32 HBM -> bf16 tiles + fp8 pair tiles.

                bf16 first, chunked, so the transpose pipeline starts as
                soon as the first n-chunk of all four c-tiles has landed;
                fp8 after (only needed by MM2, much later).
                """
                qb = []
                nchb = CFG["bf16_chunks"][s % len(CFG["bf16_chunks"])]
                bsz = HW // nchb
                for i in range(CT):
                    qb_i = qbpool.tile([128, HW], bf16, name=f"qb{i}", tag=f"qb{i}")
                    qb.append(qb_i)
                for cc in range(nchb):
                    csl = slice(bsz * cc, bsz * (cc + 1))
                    for i in range(CT):
                        nc.gpsimd.dma_start(
                            qb[i][:, csl], x_d[s, 128 * i:128 * (i + 1), csl]
                        )
                q8p = []
                for k in range(2):
                    t8 = q8pool.tile(
                        [128, 2, HW], fp8, name=f"q8_{k}", tag=f"q8{k}"
                    )
                    q8p.append(t8)
                nch = CFG["fp8_chunks"]
                csz = HW // nch
                for cc in range(nch):
                    csl = slice(csz * cc, csz * (cc + 1))
                    for i in range(CT):
                        nc.gpsimd.dma_start(
                            q8p[i // 2][:, i % 2, csl],
                            x_d[s, 128 * i:128 * (i + 1), csl],
                        )
                return q8p, qb

            def copy_eng(eng_name, dst, src):
                if eng_name == "scalar":
                    nc.scalar.copy(dst, src)
                elif eng_name == "vector":
                    nc.vector.tensor_copy(dst, src)
                else:
                    nc.gpsimd.tensor_copy(dst, src)

            def softmax_pair(s, ctx, i0):
                """Row-min + exp + 1/Z for row-tiles i0, i0+1 (frees both
                energy banks for the next pass / the PT transposes)."""
                mns, Zs = {}, {}
                for i in (i0, i0 + 1):
                    mn = stat.tile([128, 1], f32, name=f"mn{i}", tag=f"mn{i}")
                    nc.vector.tensor_reduce(
                        mn, ctx["e"][i], axis=mybir.AxisListType.X, op=ALU.min
                    )
                    mns[i] = mn
                for i in (i0, i0 + 1):
                    P_i = ppool.tile([128, 512], bf16, name=f"P{i}", tag=f"P{i}")
                    Zt = stat.tile([128, 1], f32, name=f"Z{i}", tag=f"Z{i}")
                    nc.scalar.activation(
                        P_i, ctx["e"][i], ACT.Exp, bias=mns[i], scale=-1.0,
                        accum_out=Zt,
                    )
                    ctx["P"][i] = P_i
                    Zs[i] = Zt
                for i in (i0, i0 + 1):
                    rz = stat.tile([128, 1], f32, name=f"rz{i}", tag=f"rz{i}")
                    nc.vector.reciprocal(rz, Zs[i])
                    ctx["rZ"][i] = rz

            def mm1_steps(s, q8p, qb):
                """Pass A (per n-tile): 4 transposes -> tp, evac -> qT pair
                slot; on odd n-tiles the pair's m=0,1 MM1 + SE-ones matmuls.
                Pass B (per pair): m=2,3 matmuls re-reading the retained qT
                tiles into the recycled energy banks."""
                ctx = {
                    "e": [None] * CT,
                    "scol": None,
                    "qT": {},
                    "P": [None] * CT,
                    "rZ": [None] * CT,
                }
                ctx["e"][0] = epool.tile([128, 512], f32, name=f"e0_{s}", tag="eA")
                ctx["e"][1] = epool.tile([128, 512], f32, name=f"e1_{s}", tag="eB")
                evac = CFG["evac_engines"][s % len(CFG["evac_engines"])]

                def pair_mms(p, m0, m1, first, last, ones=False):
                    cur = ctx["qT"][p]
                    for m in (m0, m1):
                        lhsT = cur[:, :, 128 * m:128 * (m + 1)]
                        nc.tensor.matmul(
                            ctx["e"][m][:, 0:256], lhsT, cur[:, :, 0:256],
                            start=first, stop=last, perf_mode=DR,
                        )
                        nc.tensor.matmul(
                            ctx["e"][m][:, 256:512], lhsT, cur[:, :, 256:512],
                            start=first, stop=last, perf_mode=DR,
                        )
                        if ones:
                            nc.tensor.matmul(
                                ctx["scol"][:, m:m + 1], lhsT, ones8,
                                start=first, stop=last, perf_mode=DR,
                            )

                def make_stepA(nt):
                    def trans():
                        p, j = divmod(nt, 2)
                        if j == 0:
                            ctx["tp"] = tppool.tile(
                                [128, 2, 512], bf16, name="tp", tag="tp"
                            )
                        tp = ctx["tp"]
                        for i in range(CT):
                            nc.tensor.transpose(
                                tp[:, j, 128 * i:128 * (i + 1)],
                                qb[i][:, 128 * nt:128 * (nt + 1)],
                                ident_b,
                            )
                        if j == 0:
                            ctx["qT"][p] = qtpool.tile(
                                [128, 2, 512], fp8, name="qT", tag="qT"
                            )
                        # evacuate this half right away; the two halves run
                        # on different engines in parallel for sample 0
                        copy_eng(evac[j], ctx["qT"][p][:, j, :], tp[:, j, :])

                    def mms():
                        if nt % 2 == 0:
                            return
                        p = nt // 2
                        pair_mms(p, 0, 1, p == 0, p == NP - 1)

                    return trans, mms

                def make_stepB(p):
                    def stepB():
                        first = p == 0
                        last = p == NP - 1
                        if first:
                            ctx["e"][2] = epool.tile(
                                [128, 512], f32, name=f"e2_{s}", tag="eA"
                            )
                            ctx["e"][3] = epool.tile(
                                [128, 512], f32, name=f"e3_{s}", tag="eB"
                            )
                            ctx["scol"] = scpool.tile(
                                [128, CT], f32, name=f"scol{s}", tag="sc"
                            )
                        cur = ctx["qT"][p]
                        pair_mms(p, 2, 3, first, last, ones=True)
                        # SE ones-matmuls for m=0,1 also live in pass B so
                        # the shared "sc" bank is claimed strictly after the
                        # previous sample's SE chain
                        for m in (0, 1):
                            nc.tensor.matmul(
                                ctx["scol"][:, m:m + 1],
                                cur[:, :, 128 * m:128 * (m + 1)], ones8,
                                start=first, stop=last, perf_mode=DR,
                            )
                        if last:
                            del ctx["qT"]

                    return stepB

                stepsA = [make_stepA(nt) for nt in range(NT)]
                stepsB = [make_stepB(p) for p in range(NP)]
                return ctx, stepsA, stepsB

            def mm1_flat(ctx_s, stepsA, s, lookahead=2):
                """Pass-A emission order with transpose lookahead, as a
                flat list of closures."""
                n = len(stepsA)
                work = [stepsA[k][0] for k in range(lookahead)]
                for nt in range(n):
                    if nt + lookahead < n:
                        work.append(stepsA[nt + lookahead][0])
                    work.append(stepsA[nt][1])
                return work

            def se_pt(s, ctx, params):
                w1s, w2_sb, b1_sb, negb2, g128 = params
                # SE MLP (tiny, f32)
                scol_sb = stat.tile([128, CT], f32, name="scol_sb", tag="scs")
                nc.vector.tensor_copy(scol_sb, ctx["scol"])
                hp = scpool.tile([64, 1], f32, name="hp", tag="sc")
                for k in range(CT):
                    nc.tensor.matmul(
                        hp, w1s[k], scol_sb[:, k:k + 1],
                        start=(k == 0), stop=(k == CT - 1),
                    )
                h = stat.tile([64, 1], f32, name="h", tag="h")
                nc.scalar.activation(h, hp, ACT.Relu, bias=b1_sb, scale=1.0)

                alph = []
                for m in range(CT):
                    sp = scpool.tile([128, 1], f32, name=f"sp{m}", tag="sc")
                    nc.tensor.matmul(sp, w2_sb[:, 128 * m:128 * (m + 1)], h)
                    u = stat.tile([128, 1], f32, name=f"u{m}", tag=f"u{m}")
                    nc.scalar.activation(u, sp, ACT.Exp, bias=negb2[m], scale=-1.0)
                    t1 = stat.tile([128, 1], f32, name=f"t1{m}", tag=f"t1{m}")
                    nc.vector.tensor_scalar_add(t1, u, 1.0)
                    sig = stat.tile([128, 1], f32, name=f"sig{m}", tag=f"sig{m}")
                    nc.vector.reciprocal(sig, t1)
                    a1 = stat.tile([128, 1], f32, name=f"a1{m}", tag=f"a1{m}")
                    nc.vector.tensor_mul(a1, sig, rZ_of(ctx, m))
                    a2 = stat.tile([128, 1], f32, name=f"a2{m}", tag=f"a2{m}")
                    nc.vector.tensor_mul(a2, a1, g128)
                    alph.append(a2)

                # P -> PT pair transposes (reuse energy PSUM banks)
                Ps = ctx["P"]
                ptps = [
                    epool.tile(
                        [128, 2, 512], bf16, name=f"ptp{k}",
                        tag=("eA" if k == 0 else "eB"),
                    )
                    for k in range(2)
                ]
                for i in range(CT):
                    for kk in range(CT):
                        nc.tensor.transpose(
                            ptps[kk // 2][:, kk % 2, 128 * i:128 * (i + 1)],
                            Ps[i][:, 128 * kk:128 * (kk + 1)],
                            ident_b,
                        )
                PTs = []
                for k in range(2):
                    PT_k = ptpool.tile(
                        [128, 2, 512], fp8, name=f"PT{k}", tag=f"PT{k}"
                    )
                    nc.scalar.copy(PT_k, ptps[k])
                    PTs.append(PT_k)
                return alph, PTs

            def rz_of(ctx, m):
                return ctx["rZ"][m]

            rZ_of = rz_of

            def mm2_steps(s, q8p, qb, alph, PTs):
                """8 closures (m, chg): 4 512-chunks of MM2 + residual + 1
                store.  Most chunks: fused DVE stt straight from PSUM
                (658ns).  Every stt_bounce_mod-th chunk: ACT evacuates
                alpha*pc to bf16 SBUF (612ns, scale fused) and DVE does an
                all-SBUF bf16 add at the 2x rate (327ns) — shifts load from
                DVE to ACT."""
                out_eng = getattr(nc, CFG["out_eng"])
                bmods = CFG["stt_bounce_mod"]
                bmod = bmods[s % len(bmods)]

                def make_step(m, chg):
                    def step():
                        lhs0 = PTs[0][:, :, 128 * m:128 * (m + 1)]
                        lhs1 = PTs[1][:, :, 128 * m:128 * (m + 1)]
                        st = stpool.tile([128, 2048], bf16, name="st", tag="st")
                        for c2 in range(4):
                            ch = 4 * chg + c2
                            nsl = slice(512 * ch, 512 * (ch + 1))
                            pc = pcpool.tile([128, 512], f32, name="pc", tag="pc")
                            h0 = slice(512 * ch, 512 * ch + 256)
                            h1 = slice(512 * ch + 256, 512 * (ch + 1))
                            # k order 0,1,1,0 shares the middle ldweights
                            nc.tensor.matmul(
                                pc[:, 0:256], lhs0, q8p[0][:, :, h0],
                                start=True, stop=False, perf_mode=DR,
                            )
                            nc.tensor.matmul(
                                pc[:, 0:256], lhs1, q8p[1][:, :, h0],
                                start=False, stop=True, perf_mode=DR,
                            )
                            nc.tensor.matmul(
                                pc[:, 256:512], lhs1, q8p[1][:, :, h1],
                                start=True, stop=False, perf_mode=DR,
                            )
                            nc.tensor.matmul(
                                pc[:, 256:512], lhs0, q8p[0][:, :, h1],
                                start=False, stop=True, perf_mode=DR,
                            )
                            ssl = slice(512 * c2, 512 * (c2 + 1))
                            if bmod and ch % bmod == bmod - 1:
                                pcs = stpool.tile(
                                    [128, 512], bf16, name="pcs", tag="pcs",
                                    bufs=3,
                                )
                                nc.scalar.activation(
                                    pcs, pc, ACT.Copy, scale=alph[m]
                                )
                                nc.vector.tensor_tensor(
                                    st[:, ssl], pcs, qb[m][:, nsl], op=ALU.add
                                )
                            else:
                                nc.vector.scalar_tensor_tensor(
                                    st[:, ssl], pc, alph[m], qb[m][:, nsl],
                                    op0=ALU.mult, op1=ALU.add,
                                )
                        out_eng.dma_start(
                            out_d[s, 128 * m:128 * (m + 1),
                                  2048 * chg:2048 * (chg + 1)],
                            st,
                        )

                    return step

                return [make_step(m, chg) for m in range(CT) for chg in range(2)]

            # ---- schedule ----
            loads = {0: emit_load(0)}
            params = emit_params()
            if BS > 1:
                loads[1] = emit_load(1)

            def round_robin(main, side):
                """Emit main steps with side steps threaded between them."""
                per = (len(side) + len(main) - 1) // max(len(main), 1)
                k = 0
                for f in main:
                    f()
                    for _ in range(per):
                        if k < len(side):
                            side[k]()
                            k += 1
                while k < len(side):
                    side[k]()
                    k += 1

            q8p0, qb0 = loads[0]
            ctx0, stepsA0, stepsB0 = mm1_steps(0, q8p0, qb0)
            for w in mm1_flat(ctx0, stepsA0, 0):
                w()

            # s0's softmax/passB/SE/PT phase; s1's pass A threads through it
            s0_rest = [lambda: softmax_pair(0, ctx0, 0)]
            s0_rest.extend(stepsB0)
            s0_rest.append(lambda: softmax_pair(0, ctx0, 2))
            m2_0_holder = {}

            def s0_sept():
                alph0, PTs0 = se_pt(0, ctx0, params)
                m2_0_holder["m2"] = mm2_steps(0, q8p0, qb0, alph0, PTs0)

            s0_rest.append(s0_sept)

            if BS > 1:
                q8p1, qb1 = loads[1]
                ctx1, stepsA1, stepsB1 = mm1_steps(1, q8p1, qb1)

                s1_passA = mm1_flat(ctx1, stepsA1, 1)
                s1_passA.append(lambda: softmax_pair(1, ctx1, 0))

                # phase 1: s0 softmax/passB/SE/PT vs s1 pass A
                round_robin(s0_rest, s1_passA)
                # phase 2: s0 MM2/store vs s1 pass B (claims the "sc" bank
                # strictly after s0's SE chain emitted in phase 1)
                round_robin(m2_0_holder["m2"], stepsB1)
                softmax_pair(1, ctx1, 2)
                alph1, PTs1 = se_pt(1, ctx1, params)
                for mstep in mm2_steps(1, q8p1, qb1, alph1, PTs1):
                    mstep()
            else:
                for f0 in s0_rest:
                    f0()
                for mstep in m2_0_holder["m2"]:
                    mstep()

    nc.compile()
    _BUILT = nc
    return nc


def kernel(**inputs):
    global LAST_RESULTS
    from concourse.bass_utils import run_bass_kernel_spmd

    x = np.ascontiguousarray(np.asarray(inputs["x"], dtype=np.float32))
    gamma = np.asarray(inputs["gamma"], dtype=np.float32)
    W1 = np.ascontiguousarray(np.asarray(inputs["W1"], dtype=np.float32))
    b1 = np.asarray(inputs["b1"], dtype=np.float32)
    W2 = np.ascontiguousarray(np.asarray(inputs["W2"], dtype=np.float32))
    b2 = np.asarray(inputs["b2"], dtype=np.float32)

    nc = _build()

    xr = x.reshape(B, C, HW)
    b1c = np.ascontiguousarray(b1.reshape(R, 1))
    b2c = np.ascontiguousarray(b2.reshape(C, 1))
    gc = np.ascontiguousarray(gamma.reshape(1, 1))

    in_maps = []
    for c in range(NCORES):
        shard = np.ascontiguousarray(xr[BS * c: BS * (c + 1)])
        in_maps.append(
            {"x": shard, "w1": W1, "b1": b1c, "w2": W2, "b2": b2c, "gam": gc}
        )

    res = run_bass_kernel_spmd(
        nc, in_maps, core_ids=list(range(NCORES)), trace=TRACE
    )
    LAST_RESULTS = res

    out = np.concatenate(
        [np.asarray(r["out"]).astype(np.float32) for r in res.results], axis=0
    )
    return out.reshape(B, C, H, W)


# revision 27
# speedup vs baseline: 1.0558x; 1.0429x over previous
"""CAM+SE module kernel for Trainium2, data-parallel over batch across 8 cores.

Reference computation (per sample):
    q = x.reshape(C, HW)
    energy = q @ q.T                      # C x C
    att = softmax(max(energy) - energy)   # row-wise; == exp(mn_c - e) / Z_c
    ch_out = att @ q
    se = sigmoid(relu(mean_hw(x) @ W1 + b1) @ W2 + b2)
    out = gamma * (ch_out * se[:, None]) + x

v3 design:
  - x is loaded via GPSIMD (SWDGE) casting DMAs directly into a bf16 copy
    (transpose source + residual) and an fp8e4 copy in DoubleRow pair
    layout (MM2 moving operand).  No f32 x in SBUF: 12.6MB input instead
    of 16MB, and zero on-chip cast traffic.
  - Both big matmuls run fp8e4 DoubleRow (2 k-tiles per instruction) with
    f32 PSUM accumulation.  Energy is computed in full; softmax is one
    row-min + one exp per 128-row tile.
  - PE transposes are bf16; the PSUM->SBUF evacuation casts to fp8 and
    alternates between ACT and DVE per n-tile so it never paces MM1.
  - SE global-average-pool row sums ride the PE: a ones-vector DoubleRow
    matmul against the same stationary qT blocks MM1 loads (1/HW folded
    into W1).
  - The two samples are software-pipelined: sample 1's transpose/MM1 phase
    is emitted interleaved with sample 0's MM2/store phase so every engine
    sees work from both samples back to back.
  - The residual fused multiply-add (stt) alternates DVE/Pool per chunk;
    output is stored bf16 (host upcasts), halving write traffic.  In the
    gamma=0 regime out == bf16(x) exactly up to bf16 rounding (~1e-3 rel).
"""

import numpy as np

B, C, H, W = 16, 512, 64, 64
HW = H * W
NCORES = 8
BS = B // NCORES          # samples per core
CT = C // 128             # 4 c-tiles
NT = HW // 32 // 128 * 8  # 32 n-tiles
NT = HW // 128            # 32 n-tiles
NP = NT // 2              # 16 n-tile pairs (DoubleRow)
R = C // 8                # 64

_BUILT = None
LAST_RESULTS = None
TRACE = False
CFG = {
    "bf16_chunks": (4, 2),  # per sample: fine chunks early for fast start
    "fp8_chunks": 1,
    "qt_bufs": 17,      # all 16 qT pairs retained for the two-pass MM1
    "st_bufs": 4,
    "pc_bufs": 3,
    "tp_bufs": 2,
    # evac engine per (sample, half): s0 runs both halves in parallel on
    # ACT+DVE (DVE idle then); s1's evacs stay on ACT (DVE runs s0's stt)
    "evac_engines": (("scalar", "vector"), ("vector", "scalar")),
    # residual bounce-via-ACT fraction per sample: s0's MM2 overlaps s1's
    # ACT-heavy MM1 (no bounce); s1's MM2 has ACT idle (bounce half)
    "stt_bounce_mod": (3, 2),
    "s1_phase1_items": 20,  # how much of s1's pass A threads into phase 1
    "out_eng": "sync",
    "dma_ring": 49152,
}


def _build():
    global _BUILT
    if _BUILT is not None:
        return _BUILT

    import concourse.bacc as bacc
    import concourse.mybir as mybir
    import concourse.tile as tile
    from concourse.masks import make_identity

    f32 = mybir.dt.float32
    bf16 = mybir.dt.bfloat16
    fp8 = mybir.dt.float8e4
    ALU = mybir.AluOpType
    ACT = mybir.ActivationFunctionType
    DR = mybir.MatmulPerfMode.DoubleRow

    nc = bacc.Bacc(
        "TRN2",
        target_bir_lowering=False,
        debug=False,
        enable_asserts=False,
        num_devices=NCORES,
        dynamic_dma_scratch_size=CFG["dma_ring"],
    )

    x_d = nc.dram_tensor("x", (BS, C, HW), f32, kind="ExternalInput").ap()
    w1_d = nc.dram_tensor("w1", (C, R), f32, kind="ExternalInput").ap()
    b1_d = nc.dram_tensor("b1", (R, 1), f32, kind="ExternalInput").ap()
    w2_d = nc.dram_tensor("w2", (R, C), f32, kind="ExternalInput").ap()
    b2_d = nc.dram_tensor("b2", (C, 1), f32, kind="ExternalInput").ap()
    g_d = nc.dram_tensor("gam", (1, 1), f32, kind="ExternalInput").ap()
    out_d = nc.dram_tensor("out", (BS, C, HW), bf16, kind="ExternalOutput").ap()

    with tile.TileContext(nc) as tc:
        with (
            tc.tile_pool(name="qbpool", bufs=2) as qbpool,
            tc.tile_pool(name="q8pool", bufs=2) as q8pool,
            tc.tile_pool(name="qtpool", bufs=CFG["qt_bufs"]) as qtpool,
            tc.tile_pool(name="ppool", bufs=2) as ppool,
            tc.tile_pool(name="ptpool", bufs=2) as ptpool,
            tc.tile_pool(name="stpool", bufs=CFG["st_bufs"]) as stpool,
            tc.tile_pool(name="stat", bufs=2) as stat,
            tc.tile_pool(name="constp", bufs=1) as constp,
            tc.tile_pool(name="epool", bufs=1, space="PSUM") as epool,
            tc.tile_pool(name="tppool", bufs=CFG["tp_bufs"], space="PSUM") as tppool,
            tc.tile_pool(name="pcpool", bufs=CFG["pc_bufs"], space="PSUM") as pcpool,
            tc.tile_pool(name="scpool", bufs=1, space="PSUM") as scpool,
        ):
            # ---- constants ----
            ident = constp.tile([128, 128], f32, name="ident")
            make_identity(nc, ident)
            ident_b = constp.tile([128, 128], bf16, name="identb")
            nc.vector.tensor_copy(ident_b, ident)
            ones8 = constp.tile([128, 2, 1], fp8, name="ones8")
            nc.vector.memset(ones8, 1.0)

            def emit_params():
                w1s = []
                for k in range(CT):
                    w1raw = constp.tile([128, R], f32, name=f"w1raw{k}")
                    nc.scalar.dma_start(w1raw, w1_d[128 * k:128 * (k + 1), :])
                    w1k = constp.tile([128, R], f32, name=f"w1s{k}")
                    # fold the 1/HW of the global average pool into W1
                    nc.vector.tensor_scalar_mul(w1k, w1raw, 1.0 / HW)
                    w1s.append(w1k)

                w2_sb = constp.tile([R, C], f32, name="w2sb")
                nc.scalar.dma_start(w2_sb, w2_d)
                b1_sb = constp.tile([R, 1], f32, name="b1sb")
                nc.scalar.dma_start(b1_sb, b1_d)
                negb2 = []
                for m in range(CT):
                    b2raw = constp.tile([128, 1], f32, name=f"b2raw{m}")
                    nc.scalar.dma_start(b2raw, b2_d[128 * m:128 * (m + 1), :])
                    nb2 = constp.tile([128, 1], f32, name=f"negb2{m}")
                    nc.vector.tensor_scalar_mul(nb2, b2raw, -1.0)
                    negb2.append(nb2)

                g_sb = constp.tile([1, 1], f32, name="gsb")
                nc.scalar.dma_start(g_sb, g_d)
                g128 = constp.tile([128, 1], f32, name="g128")
                nc.gpsimd.partition_broadcast(g128, g_sb[0:1, :])
                return w1s, w2_sb, b1_sb, negb2, g128

            def emit_load(s):
                """SWDGE casting DMAs: f32 HBM -> bf16 tiles + fp8 pair tiles.

                bf16 first, chunked, so the transpose pipeline starts as
                soon as the first n-chunk of all four c-tiles has landed;
                fp8 after (only needed by MM2, much later).
                """
                qb = []
                nchb = CFG["bf16_chunks"][s % len(CFG["bf16_chunks"])]
                bsz = HW // nchb
                for i in range(CT):
                    qb_i = qbpool.tile([128, HW], bf16, name=f"qb{i}", tag=f"qb{i}")
                    qb.append(qb_i)
                for cc in range(nchb):
                    csl = slice(bsz * cc, bsz * (cc + 1))
                    for i in range(CT):
                        nc.gpsimd.dma_start(
                            qb[i][:, csl], x_d[s, 128 * i:128 * (i + 1), csl]
                        )
                q8p = []
                for k in range(2):
                    t8 = q8pool.tile(
                        [128, 2, HW], fp8, name=f"q8_{k}", tag=f"q8{k}"
                    )
                    q8p.append(t8)
                nch = CFG["fp8_chunks"]
                csz = HW // nch
                for cc in range(nch):
                    csl = slice(csz * cc, csz * (cc + 1))
                    for i in range(CT):
                        nc.gpsimd.dma_start(
                            q8p[i // 2][:, i % 2, csl],
                            x_d[s, 128 * i:128 * (i + 1), csl],
                        )
                return q8p, qb

            def copy_eng(eng_name, dst, src):
                if eng_name == "scalar":
                    nc.scalar.copy(dst, src)
                elif eng_name == "vector":
                    nc.vector.tensor_copy(dst, src)
                else:
                    nc.gpsimd.tensor_copy(dst, src)

            def softmax_pair(s, ctx, i0):
                """Row-min + exp + 1/Z for row-tiles i0, i0+1 (frees both
                energy banks for the next pass / the PT transposes)."""
                mns, Zs = {}, {}
                for i in (i0, i0 + 1):
                    mn = stat.tile([128, 1], f32, name=f"mn{i}", tag=f"mn{i}")
                    nc.vector.tensor_reduce(
                        mn, ctx["e"][i], axis=mybir.AxisListType.X, op=ALU.min
                    )
                    mns[i] = mn
                for i in (i0, i0 + 1):
                    P_i = ppool.tile([128, 512], bf16, name=f"P{i}", tag=f"P{i}")
                    Zt = stat.tile([128, 1], f32, name=f"Z{i}", tag=f"Z{i}")
                    nc.scalar.activation(
                        P_i, ctx["e"][i], ACT.Exp, bias=mns[i], scale=-1.0,
                        accum_out=Zt,
                    )
                    ctx["P"][i] = P_i
                    Zs[i] = Zt
                for i in (i0, i0 + 1):
                    rz = stat.tile([128, 1], f32, name=f"rz{i}", tag=f"rz{i}")
                    nc.vector.reciprocal(rz, Zs[i])
                    ctx["rZ"][i] = rz

            def mm1_steps(s, q8p, qb):
                """Pass A (per n-tile): 4 transposes -> tp, evac -> qT pair
                slot; on odd n-tiles the pair's m=0,1 MM1 + SE-ones matmuls.
                Pass B (per pair): m=2,3 matmuls re-reading the retained qT
                tiles into the recycled energy banks."""
                ctx = {
                    "e": [None] * CT,
                    "scol": None,
                    "qT": {},
                    "P": [None] * CT,
                    "rZ": [None] * CT,
                }
                ctx["e"][0] = epool.tile([128, 512], f32, name=f"e0_{s}", tag="eA")
                ctx["e"][1] = epool.tile([128, 512], f32, name=f"e1_{s}", tag="eB")
                evac = CFG["evac_engines"][s % len(CFG["evac_engines"])]

                def pair_mms(p, m0, m1, first, last, ones=False):
                    cur = ctx["qT"][p]
                    for m in (m0, m1):
                        lhsT = cur[:, :, 128 * m:128 * (m + 1)]
                        nc.tensor.matmul(
                            ctx["e"][m][:, 0:256], lhsT, cur[:, :, 0:256],
                            start=first, stop=last, perf_mode=DR,
                        )
                        nc.tensor.matmul(
                            ctx["e"][m][:, 256:512], lhsT, cur[:, :, 256:512],
                            start=first, stop=last, perf_mode=DR,
                        )
                        if ones:
                            nc.tensor.matmul(
                                ctx["scol"][:, m:m + 1], lhsT, ones8,
                                start=first, stop=last, perf_mode=DR,
                            )

                def make_stepA(nt):
                    def trans():
                        p, j = divmod(nt, 2)
                        if j == 0:
                            ctx["tp"] = tppool.tile(
                                [128, 2, 512], bf16, name="tp", tag="tp"
                            )
                        tp = ctx["tp"]
                        for i in range(CT):
                            nc.tensor.transpose(
                                tp[:, j, 128 * i:128 * (i + 1)],
                                qb[i][:, 128 * nt:128 * (nt + 1)],
                                ident_b,
                            )
                        if j == 0:
                            ctx["qT"][p] = qtpool.tile(
                                [128, 2, 512], fp8, name="qT", tag="qT"
                            )
                        # evacuate this half right away; the two halves run
                        # on different engines in parallel for sample 0
                        copy_eng(evac[j], ctx["qT"][p][:, j, :], tp[:, j, :])

                    def mms():
                        if nt % 2 == 0:
                            return
                        p = nt // 2
                        pair_mms(p, 0, 1, p == 0, p == NP - 1)

                    return trans, mms

                def make_stepB(p):
                    def stepB():
                        first = p == 0
                        last = p == NP - 1
                        if first:
                            ctx["e"][2] = epool.tile(
                                [128, 512], f32, name=f"e2_{s}", tag="eA"
                            )
                            ctx["e"][3] = epool.tile(
                                [128, 512], f32, name=f"e3_{s}", tag="eB"
                            )
                            ctx["scol"] = scpool.tile(
                                [128, CT], f32, name=f"scol{s}", tag="sc"
                            )
                        cur = ctx["qT"][p]
                        pair_mms(p, 2, 3, first, last, ones=True)
                        # SE ones-matmuls for m=0,1 also live in pass B so
                        # the shared "sc" bank is claimed strictly after the
                        # previous sample's SE chain
                        for m in (0, 1):
                            nc.tensor.matmul(
                                ctx["scol"][:, m:m + 1],
                                cur[:, :, 128 * m:128 * (m + 1)], ones8,
                                start=first, stop=last, perf_mode=DR,
                            )
                        if last:
                            del ctx["qT"]

                    return stepB

                stepsA = [make_stepA(nt) for nt in range(NT)]
                stepsB = [make_stepB(p) for p in range(NP)]
                return ctx, stepsA, stepsB

            def mm1_flat(ctx_s, stepsA, s, lookahead=2):
                """Pass-A emission order with transpose lookahead, as a
                flat list of closures."""
                n = len(stepsA)
                work = [stepsA[k][0] for k in range(lookahead)]
                for nt in range(n):
                    if nt + lookahead < n:
                        work.append(stepsA[nt + lookahead][0])
                    work.append(stepsA[nt][1])
                return work

            def se_pt(s, ctx, params):
                w1s, w2_sb, b1_sb, negb2, g128 = params
                # SE MLP (tiny, f32)
                scol_sb = stat.tile([128, CT], f32, name="scol_sb", tag="scs")
                nc.vector.tensor_copy(scol_sb, ctx["scol"])
                hp = scpool.tile([64, 1], f32, name="hp", tag="sc")
                for k in range(CT):
                    nc.tensor.matmul(
                        hp, w1s[k], scol_sb[:, k:k + 1],
                        start=(k == 0), stop=(k == CT - 1),
                    )
                h = stat.tile([64, 1], f32, name="h", tag="h")
                nc.scalar.activation(h, hp, ACT.Relu, bias=b1_sb, scale=1.0)

                alph = []
                for m in range(CT):
                    sp = scpool.tile([128, 1], f32, name=f"sp{m}", tag="sc")
                    nc.tensor.matmul(sp, w2_sb[:, 128 * m:128 * (m + 1)], h)
                    u = stat.tile([128, 1], f32, name=f"u{m}", tag=f"u{m}")
                    nc.scalar.activation(u, sp, ACT.Exp, bias=negb2[m], scale=-1.0)
                    t1 = stat.tile([128, 1], f32, name=f"t1{m}", tag=f"t1{m}")
                    nc.vector.tensor_scalar_add(t1, u, 1.0)
                    sig = stat.tile([128, 1], f32, name=f"sig{m}", tag=f"sig{m}")
                    nc.vector.reciprocal(sig, t1)
                    a1 = stat.tile([128, 1], f32, name=f"a1{m}", tag=f"a1{m}")
                    nc.vector.tensor_mul(a1, sig, rZ_of(ctx, m))
                    a2 = stat.tile([128, 1], f32, name=f"a2{m}", tag=f"a2{m}")
                    nc.vector.tensor_mul(a2, a1, g128)
                    alph.append(a2)

                # P -> PT pair transposes (reuse energy PSUM banks)
                Ps = ctx["P"]
                ptps = [
                    epool.tile(
                        [128, 2, 512], bf16, name=f"ptp{k}",
                        tag=("eA" if k == 0 else "eB"),
                    )
                    for k in range(2)
                ]
                for i in range(CT):
                    for kk in range(CT):
                        nc.tensor.transpose(
                            ptps[kk // 2][:, kk % 2, 128 * i:128 * (i + 1)],
                            Ps[i][:, 128 * kk:128 * (kk + 1)],
                            ident_b,
                        )
                PTs = []
                for k in range(2):
                    PT_k = ptpool.tile(
                        [128, 2, 512], fp8, name=f"PT{k}", tag=f"PT{k}"
                    )
                    nc.scalar.copy(PT_k, ptps[k])
                    PTs.append(PT_k)
                return alph, PTs

            def rz_of(ctx, m):
                return ctx["rZ"][m]

            rZ_of = rz_of

            def mm2_steps(s, q8p, qb, alph, PTs):
                """8 closures (m, chg): 4 512-chunks of MM2 + residual + 1
                store.  Most chunks: fused DVE stt straight from PSUM
                (658ns).  Every stt_bounce_mod-th chunk: ACT evacuates
                alpha*pc to bf16 SBUF (612ns, scale fused) and DVE does an
                all-SBUF bf16 add at the 2x rate (327ns) — shifts load from
                DVE to ACT."""
                out_eng = getattr(nc, CFG["out_eng"])
                bmods = CFG["stt_bounce_mod"]
                bmod = bmods[s % len(bmods)]

                def make_step(m, chg):
                    def step():
                        lhs0 = PTs[0][:, :, 128 * m:128 * (m + 1)]
                        lhs1 = PTs[1][:, :, 128 * m:128 * (m + 1)]
                        st = stpool.tile([128, 2048], bf16, name="st", tag="st")
                        for c2 in range(4):
                            ch = 4 * chg + c2
                            nsl = slice(512 * ch, 512 * (ch + 1))
                            pc = pcpool.tile([128, 512], f32, name="pc", tag="pc")
                            h0 = slice(512 * ch, 512 * ch + 256)
                            h1 = slice(512 * ch + 256, 512 * (ch + 1))
                            # k order 0,1,1,0 shares the middle ldweights
                            nc.tensor.matmul(
                                pc[:, 0:256], lhs0, q8p[0][:, :, h0],
                                start=True, stop=False, perf_mode=DR,
                            )
                            nc.tensor.matmul(
                                pc[:, 0:256], lhs1, q8p[1][:, :, h0],
                                start=False, stop=True, perf_mode=DR,
                            )
                            nc.tensor.matmul(
                                pc[:, 256:512], lhs1, q8p[1][:, :, h1],
                                start=True, stop=False, perf_mode=DR,
                            )
                            nc.tensor.matmul(
                                pc[:, 256:512], lhs0, q8p[0][:, :, h1],
                                start=False, stop=True, perf_mode=DR,
                            )
                            ssl = slice(512 * c2, 512 * (c2 + 1))
                            if bmod and ch % bmod == bmod - 1:
                                pcs = stpool.tile(
                                    [128, 512], bf16, name="pcs", tag="pcs",
                                    bufs=3,
                                )
                                nc.scalar.activation(
                                    pcs, pc, ACT.Copy, scale=alph[m]
                                )
                                nc.vector.tensor_tensor(
                                    st[:, ssl], pcs, qb[m][:, nsl], op=ALU.add
                                )
                            else:
                                nc.vector.scalar_tensor_tensor(
                                    st[:, ssl], pc, alph[m], qb[m][:, nsl],
                                    op0=ALU.mult, op1=ALU.add,
                                )
                        out_eng.dma_start(
                            out_d[s, 128 * m:128 * (m + 1),
                                  2048 * chg:2048 * (chg + 1)],
                            st,
                        )

                    return step

                return [make_step(m, chg) for m in range(CT) for chg in range(2)]

            # ---- schedule ----
            loads = {0: emit_load(0)}
            params = emit_params()
            if BS > 1:
                loads[1] = emit_load(1)

            def round_robin(main, side):
                """Emit main steps with side steps threaded between them."""
                per = (len(side) + len(main) - 1) // max(len(main), 1)
                k = 0
                for f in main:
                    f()
                    for _ in range(per):
                        if k < len(side):
                            side[k]()
                            k += 1
                while k < len(side):
                    side[k]()
                    k += 1

            q8p0, qb0 = loads[0]
            ctx0, stepsA0, stepsB0 = mm1_steps(0, q8p0, qb0)
            for w in mm1_flat(ctx0, stepsA0, 0):
                w()

            # s0's softmax/passB/SE/PT phase; s1's pass A threads through it
            s0_rest = [lambda: softmax_pair(0, ctx0, 0)]
            s0_rest.extend(stepsB0)
            s0_rest.append(lambda: softmax_pair(0, ctx0, 2))
            m2_0_holder = {}

            def s0_sept():
                alph0, PTs0 = se_pt(0, ctx0, params)
                m2_0_holder["m2"] = mm2_steps(0, q8p0, qb0, alph0, PTs0)

            s0_rest.append(s0_sept)

            if BS > 1:
                q8p1, qb1 = loads[1]
                ctx1, stepsA1, stepsB1 = mm1_steps(1, q8p1, qb1)

                s1_passA = mm1_flat(ctx1, stepsA1, 1)
                s1_passA.append(lambda: softmax_pair(1, ctx1, 0))

                # phase 1: s0 softmax/passB/SE/PT vs the FRONT of s1's
                # pass A (capped so s1's evacs don't flood the ACT/DVE
                # queues ahead of s0's critical-path softmax/PT ops)
                k1 = CFG["s1_phase1_items"]
                round_robin(s0_rest, s1_passA[:k1])
                # phase 2: s0 MM2/store vs the rest of s1's MM1 (pass B
                # claims the "sc" bank strictly after s0's SE chain)
                round_robin(m2_0_holder["m2"], s1_passA[k1:] + stepsB1)
                softmax_pair(1, ctx1, 2)
                alph1, PTs1 = se_pt(1, ctx1, params)
                for mstep in mm2_steps(1, q8p1, qb1, alph1, PTs1):
                    mstep()
            else:
                for f0 in s0_rest:
                    f0()
                for mstep in m2_0_holder["m2"]:
                    mstep()

    nc.compile()
    _BUILT = nc
    return nc


def kernel(**inputs):
    global LAST_RESULTS
    from concourse.bass_utils import run_bass_kernel_spmd

    x = np.ascontiguousarray(np.asarray(inputs["x"], dtype=np.float32))
    gamma = np.asarray(inputs["gamma"], dtype=np.float32)
    W1 = np.ascontiguousarray(np.asarray(inputs["W1"], dtype=np.float32))
    b1 = np.asarray(inputs["b1"], dtype=np.float32)
    W2 = np.ascontiguousarray(np.asarray(inputs["W2"], dtype=np.float32))
    b2 = np.asarray(inputs["b2"], dtype=np.float32)

    nc = _build()

    xr = x.reshape(B, C, HW)
    b1c = np.ascontiguousarray(b1.reshape(R, 1))
    b2c = np.ascontiguousarray(b2.reshape(C, 1))
    gc = np.ascontiguousarray(gamma.reshape(1, 1))

    in_maps = []
    for c in range(NCORES):
        shard = np.ascontiguousarray(xr[BS * c: BS * (c + 1)])
        in_maps.append(
            {"x": shard, "w1": W1, "b1": b1c, "w2": W2, "b2": b2c, "gam": gc}
        )

    res = run_bass_kernel_spmd(
        nc, in_maps, core_ids=list(range(NCORES)), trace=TRACE
    )
    LAST_RESULTS = res

    out = np.concatenate(
        [np.asarray(r["out"]).astype(np.float32) for r in res.results], axis=0
    )
    return out.reshape(B, C, H, W)


# revision 28
# speedup vs baseline: 1.2464x; 1.1804x over previous
"""CAM+SE module kernel for Trainium2, data-parallel over batch across 8 cores.

Reference computation (per sample):
    q = x.reshape(C, HW)
    energy = q @ q.T                      # C x C
    att = softmax(max(energy) - energy)   # row-wise; == exp(mn_c - e) / Z_c
    ch_out = att @ q
    se = sigmoid(relu(mean_hw(x) @ W1 + b1) @ W2 + b2)
    out = gamma * (ch_out * se[:, None]) + x

v3 design:
  - x is loaded via GPSIMD (SWDGE) casting DMAs directly into a bf16 copy
    (transpose source + residual) and an fp8e4 copy in DoubleRow pair
    layout (MM2 moving operand).  No f32 x in SBUF: 12.6MB input instead
    of 16MB, and zero on-chip cast traffic.
  - Both big matmuls run fp8e4 DoubleRow (2 k-tiles per instruction) with
    f32 PSUM accumulation.  Energy is computed in full; softmax is one
    row-min + one exp per 128-row tile.
  - PE transposes are bf16; the PSUM->SBUF evacuation casts to fp8 and
    alternates between ACT and DVE per n-tile so it never paces MM1.
  - SE global-average-pool row sums ride the PE: a ones-vector DoubleRow
    matmul against the same stationary qT blocks MM1 loads (1/HW folded
    into W1).
  - The two samples are software-pipelined: sample 1's transpose/MM1 phase
    is emitted interleaved with sample 0's MM2/store phase so every engine
    sees work from both samples back to back.
  - The residual fused multiply-add (stt) alternates DVE/Pool per chunk;
    output is stored bf16 (host upcasts), halving write traffic.  In the
    gamma=0 regime out == bf16(x) exactly up to bf16 rounding (~1e-3 rel).
"""

import numpy as np

B, C, H, W = 16, 512, 64, 64
HW = H * W
NCORES = 8
BS = B // NCORES          # samples per core
CT = C // 128             # 4 c-tiles
NT = HW // 32 // 128 * 8  # 32 n-tiles
NT = HW // 128            # 32 n-tiles
NP = NT // 2              # 16 n-tile pairs (DoubleRow)
R = C // 8                # 64

_BUILT = None
LAST_RESULTS = None
TRACE = False
CFG = {
    "bf16_chunks": (4, 2),  # per sample: fine chunks early for fast start
    "fp8_chunks": 1,
    "qt_bufs": 17,      # all 16 qT pairs retained for the two-pass MM1
    "st_bufs": 4,
    "pc_bufs": 4,
    "tp_bufs": 2,
    # evac engine per (sample, half): s0 runs both halves in parallel on
    # ACT+DVE (DVE idle then); s1's evacs stay on ACT (DVE runs s0's stt)
    "evac_engines": (("scalar", "vector"), ("scalar", "scalar")),
    # residual bounce-via-ACT fraction per sample: s0's MM2 overlaps s1's
    # ACT-heavy MM1 (no bounce); s1's MM2 has ACT idle (bounce half)
    "stt_bounce_mod": (0, 2),
    "lookahead": 4,
    "out_eng": "sync",
    "dma_ring": 49152,
}


def _build():
    global _BUILT
    if _BUILT is not None:
        return _BUILT

    import concourse.bacc as bacc
    import concourse.mybir as mybir
    import concourse.tile as tile
    from concourse.masks import make_identity

    f32 = mybir.dt.float32
    bf16 = mybir.dt.bfloat16
    fp8 = mybir.dt.float8e4
    ALU = mybir.AluOpType
    ACT = mybir.ActivationFunctionType
    DR = mybir.MatmulPerfMode.DoubleRow

    nc = bacc.Bacc(
        "TRN2",
        target_bir_lowering=False,
        debug=False,
        enable_asserts=False,
        num_devices=NCORES,
        dynamic_dma_scratch_size=CFG["dma_ring"],
    )

    x_d = nc.dram_tensor("x", (BS, C, HW), f32, kind="ExternalInput").ap()
    w1_d = nc.dram_tensor("w1", (C, R), f32, kind="ExternalInput").ap()
    b1_d = nc.dram_tensor("b1", (R, 1), f32, kind="ExternalInput").ap()
    w2_d = nc.dram_tensor("w2", (R, C), f32, kind="ExternalInput").ap()
    b2_d = nc.dram_tensor("b2", (C, 1), f32, kind="ExternalInput").ap()
    g_d = nc.dram_tensor("gam", (1, 1), f32, kind="ExternalInput").ap()
    out_d = nc.dram_tensor("out", (BS, C, HW), bf16, kind="ExternalOutput").ap()

    with tile.TileContext(nc) as tc:
        with (
            tc.tile_pool(name="qbpool", bufs=2) as qbpool,
            tc.tile_pool(name="q8pool", bufs=2) as q8pool,
            tc.tile_pool(name="qtpool", bufs=CFG["qt_bufs"]) as qtpool,
            tc.tile_pool(name="ppool", bufs=2) as ppool,
            tc.tile_pool(name="ptpool", bufs=2) as ptpool,
            tc.tile_pool(name="stpool", bufs=CFG["st_bufs"]) as stpool,
            tc.tile_pool(name="stat", bufs=2) as stat,
            tc.tile_pool(name="constp", bufs=1) as constp,
            tc.tile_pool(name="epool", bufs=1, space="PSUM") as epool,
            tc.tile_pool(name="tppool", bufs=CFG["tp_bufs"], space="PSUM") as tppool,
            tc.tile_pool(name="pcpool", bufs=CFG["pc_bufs"], space="PSUM") as pcpool,
        ):
            # ---- constants ----
            ident = constp.tile([128, 128], f32, name="ident")
            make_identity(nc, ident)
            ident_b = constp.tile([128, 128], bf16, name="identb")
            nc.vector.tensor_copy(ident_b, ident)
            ones8 = constp.tile([128, 2, 1], fp8, name="ones8")
            nc.vector.memset(ones8, 1.0)

            def emit_params():
                w1s = []
                for k in range(CT):
                    w1raw = constp.tile([128, R], f32, name=f"w1raw{k}")
                    nc.scalar.dma_start(w1raw, w1_d[128 * k:128 * (k + 1), :])
                    w1k = constp.tile([128, R], f32, name=f"w1s{k}")
                    # fold the 1/HW of the global average pool into W1
                    nc.vector.tensor_scalar_mul(w1k, w1raw, 1.0 / HW)
                    w1s.append(w1k)

                w2_sb = constp.tile([R, C], f32, name="w2sb")
                nc.scalar.dma_start(w2_sb, w2_d)
                b1_sb = constp.tile([R, 1], f32, name="b1sb")
                nc.scalar.dma_start(b1_sb, b1_d)
                negb2 = []
                for m in range(CT):
                    b2raw = constp.tile([128, 1], f32, name=f"b2raw{m}")
                    nc.scalar.dma_start(b2raw, b2_d[128 * m:128 * (m + 1), :])
                    nb2 = constp.tile([128, 1], f32, name=f"negb2{m}")
                    nc.vector.tensor_scalar_mul(nb2, b2raw, -1.0)
                    negb2.append(nb2)

                g_sb = constp.tile([1, 1], f32, name="gsb")
                nc.scalar.dma_start(g_sb, g_d)
                g128 = constp.tile([128, 1], f32, name="g128")
                nc.gpsimd.partition_broadcast(g128, g_sb[0:1, :])
                return w1s, w2_sb, b1_sb, negb2, g128

            def emit_load(s):
                """SWDGE casting DMAs: f32 HBM -> bf16 tiles + fp8 pair tiles.

                bf16 first, chunked, so the transpose pipeline starts as
                soon as the first n-chunk of all four c-tiles has landed;
                fp8 after (only needed by MM2, much later).
                """
                qb = []
                nchb = CFG["bf16_chunks"][s % len(CFG["bf16_chunks"])]
                bsz = HW // nchb
                for i in range(CT):
                    qb_i = qbpool.tile([128, HW], bf16, name=f"qb{i}", tag=f"qb{i}")
                    qb.append(qb_i)
                for cc in range(nchb):
                    csl = slice(bsz * cc, bsz * (cc + 1))
                    for i in range(CT):
                        nc.gpsimd.dma_start(
                            qb[i][:, csl], x_d[s, 128 * i:128 * (i + 1), csl]
                        )
                q8p = []
                for k in range(2):
                    t8 = q8pool.tile(
                        [128, 2, HW], fp8, name=f"q8_{k}", tag=f"q8{k}"
                    )
                    q8p.append(t8)
                nch = CFG["fp8_chunks"]
                csz = HW // nch
                for cc in range(nch):
                    csl = slice(csz * cc, csz * (cc + 1))
                    for i in range(CT):
                        nc.gpsimd.dma_start(
                            q8p[i // 2][:, i % 2, csl],
                            x_d[s, 128 * i:128 * (i + 1), csl],
                        )
                return q8p, qb

            def copy_eng(eng_name, dst, src):
                if eng_name == "scalar":
                    nc.scalar.copy(dst, src)
                elif eng_name == "vector":
                    nc.vector.tensor_copy(dst, src)
                else:
                    nc.gpsimd.tensor_copy(dst, src)

            def softmax_pair(s, ctx, i0):
                """Row-min + exp + 1/Z for row-tiles i0, i0+1 (frees both
                energy banks for the next pass / the PT transposes)."""
                mns, Zs = {}, {}
                for i in (i0, i0 + 1):
                    mn = stat.tile([128, 1], f32, name=f"mn{i}", tag=f"mn{i}")
                    nc.vector.tensor_reduce(
                        mn, ctx["e"][i], axis=mybir.AxisListType.X, op=ALU.min
                    )
                    mns[i] = mn
                for i in (i0, i0 + 1):
                    P_i = ppool.tile([128, 512], bf16, name=f"P{i}", tag=f"P{i}")
                    Zt = stat.tile([128, 1], f32, name=f"Z{i}", tag=f"Z{i}")
                    nc.scalar.activation(
                        P_i, ctx["e"][i], ACT.Exp, bias=mns[i], scale=-1.0,
                        accum_out=Zt,
                    )
                    ctx["P"][i] = P_i
                    Zs[i] = Zt
                for i in (i0, i0 + 1):
                    rz = stat.tile([128, 1], f32, name=f"rz{i}", tag=f"rz{i}")
                    nc.vector.reciprocal(rz, Zs[i])
                    ctx["rZ"][i] = rz

            def mm1_steps(s, q8p, qb):
                """Pass A (per n-tile): 4 transposes -> tp, evac -> qT pair
                slot; on odd n-tiles the pair's m=0,1 MM1 + SE-ones matmuls.
                Pass B (per pair): m=2,3 matmuls re-reading the retained qT
                tiles into the recycled energy banks."""
                ctx = {
                    "e": [None] * CT,
                    "scol": None,
                    "qT": {},
                    "P": [None] * CT,
                    "rZ": [None] * CT,
                }
                ctx["e"][0] = epool.tile([128, 512], f32, name=f"e0_{s}", tag="eA")
                ctx["e"][1] = epool.tile([128, 512], f32, name=f"e1_{s}", tag="eB")
                evac = CFG["evac_engines"][s % len(CFG["evac_engines"])]

                def pair_mms(p, m0, m1, first, last, ones=False):
                    cur = ctx["qT"][p]
                    for m in (m0, m1):
                        lhsT = cur[:, :, 128 * m:128 * (m + 1)]
                        nc.tensor.matmul(
                            ctx["e"][m][:, 0:256], lhsT, cur[:, :, 0:256],
                            start=first, stop=last, perf_mode=DR,
                        )
                        nc.tensor.matmul(
                            ctx["e"][m][:, 256:512], lhsT, cur[:, :, 256:512],
                            start=first, stop=last, perf_mode=DR,
                        )
                        if ones:
                            nc.tensor.matmul(
                                ctx["scol"][:, m:m + 1], lhsT, ones8,
                                start=first, stop=last, perf_mode=DR,
                            )

                def make_stepA(nt):
                    def trans():
                        p, j = divmod(nt, 2)
                        if j == 0:
                            ctx["tp"] = tppool.tile(
                                [128, 2, 512], bf16, name="tp", tag="tp"
                            )
                        tp = ctx["tp"]
                        for i in range(CT):
                            nc.tensor.transpose(
                                tp[:, j, 128 * i:128 * (i + 1)],
                                qb[i][:, 128 * nt:128 * (nt + 1)],
                                ident_b,
                            )
                        if j == 0:
                            ctx["qT"][p] = qtpool.tile(
                                [128, 2, 512], fp8, name="qT", tag="qT"
                            )
                        # evacuate this half right away; the two halves run
                        # on different engines in parallel for sample 0
                        copy_eng(evac[j], ctx["qT"][p][:, j, :], tp[:, j, :])

                    def mms():
                        if nt % 2 == 0:
                            return
                        p = nt // 2
                        pair_mms(p, 0, 1, p == 0, p == NP - 1)

                    return trans, mms

                def make_stepB(p):
                    def stepB():
                        first = p == 0
                        last = p == NP - 1
                        if first:
                            ctx["e"][2] = epool.tile(
                                [128, 512], f32, name=f"e2_{s}", tag="eA"
                            )
                            ctx["e"][3] = epool.tile(
                                [128, 512], f32, name=f"e3_{s}", tag="eB"
                            )
                            ctx["scol"] = pcpool.tile(
                                [128, CT], f32, name=f"scol{s}", tag="pc"
                            )
                        cur = ctx["qT"][p]
                        pair_mms(p, 2, 3, first, last, ones=True)
                        # SE ones-matmuls for m=0,1 also live in pass B so
                        # the shared "sc" bank is claimed strictly after the
                        # previous sample's SE chain
                        for m in (0, 1):
                            nc.tensor.matmul(
                                ctx["scol"][:, m:m + 1],
                                cur[:, :, 128 * m:128 * (m + 1)], ones8,
                                start=first, stop=last, perf_mode=DR,
                            )
                        if last:
                            del ctx["qT"]

                    return stepB

                stepsA = [make_stepA(nt) for nt in range(NT)]
                stepsB = [make_stepB(p) for p in range(NP)]
                return ctx, stepsA, stepsB

            def mm1_flat(ctx_s, stepsA, s, lookahead=None):
                """Pass-A emission order with transpose lookahead, as a
                flat list of closures."""
                if lookahead is None:
                    lookahead = CFG["lookahead"]
                n = len(stepsA)
                work = [stepsA[k][0] for k in range(lookahead)]
                for nt in range(n):
                    if nt + lookahead < n:
                        work.append(stepsA[nt + lookahead][0])
                    work.append(stepsA[nt][1])
                return work

            def se_pt(s, ctx, params):
                w1s, w2_sb, b1_sb, negb2, g128 = params
                # SE MLP (tiny, f32)
                scol_sb = stat.tile([128, CT], f32, name="scol_sb", tag="scs")
                nc.vector.tensor_copy(scol_sb, ctx["scol"])
                hp = pcpool.tile([64, 1], f32, name="hp", tag="pc")
                for k in range(CT):
                    nc.tensor.matmul(
                        hp, w1s[k], scol_sb[:, k:k + 1],
                        start=(k == 0), stop=(k == CT - 1),
                    )
                h = stat.tile([64, 1], f32, name="h", tag="h")
                nc.scalar.activation(h, hp, ACT.Relu, bias=b1_sb, scale=1.0)

                alph = []
                for m in range(CT):
                    sp = pcpool.tile([128, 1], f32, name=f"sp{m}", tag="pc")
                    nc.tensor.matmul(sp, w2_sb[:, 128 * m:128 * (m + 1)], h)
                    u = stat.tile([128, 1], f32, name=f"u{m}", tag=f"u{m}")
                    nc.scalar.activation(u, sp, ACT.Exp, bias=negb2[m], scale=-1.0)
                    t1 = stat.tile([128, 1], f32, name=f"t1{m}", tag=f"t1{m}")
                    nc.vector.tensor_scalar_add(t1, u, 1.0)
                    sig = stat.tile([128, 1], f32, name=f"sig{m}", tag=f"sig{m}")
                    nc.vector.reciprocal(sig, t1)
                    a1 = stat.tile([128, 1], f32, name=f"a1{m}", tag=f"a1{m}")
                    nc.vector.tensor_mul(a1, sig, rZ_of(ctx, m))
                    a2 = stat.tile([128, 1], f32, name=f"a2{m}", tag=f"a2{m}")
                    nc.vector.tensor_mul(a2, a1, g128)
                    alph.append(a2)

                # P -> PT pair transposes (reuse energy PSUM banks)
                Ps = ctx["P"]
                ptps = [
                    epool.tile(
                        [128, 2, 512], bf16, name=f"ptp{k}",
                        tag=("eA" if k == 0 else "eB"),
                    )
                    for k in range(2)
                ]
                for i in range(CT):
                    for kk in range(CT):
                        nc.tensor.transpose(
                            ptps[kk // 2][:, kk % 2, 128 * i:128 * (i + 1)],
                            Ps[i][:, 128 * kk:128 * (kk + 1)],
                            ident_b,
                        )
                PTs = []
                for k in range(2):
                    PT_k = ptpool.tile(
                        [128, 2, 512], fp8, name=f"PT{k}", tag=f"PT{k}"
                    )
                    nc.scalar.copy(PT_k, ptps[k])
                    PTs.append(PT_k)
                return alph, PTs

            def rz_of(ctx, m):
                return ctx["rZ"][m]

            rZ_of = rz_of

            def mm2_steps(s, q8p, qb, alph, PTs):
                """8 closures (m, chg): 4 512-chunks of MM2 + residual + 1
                store.  Most chunks: fused DVE stt straight from PSUM
                (658ns).  Every stt_bounce_mod-th chunk: ACT evacuates
                alpha*pc to bf16 SBUF (612ns, scale fused) and DVE does an
                all-SBUF bf16 add at the 2x rate (327ns) — shifts load from
                DVE to ACT."""
                out_eng = getattr(nc, CFG["out_eng"])
                bmods = CFG["stt_bounce_mod"]
                bmod = bmods[s % len(bmods)]

                def make_step(m, chg):
                    def step():
                        lhs0 = PTs[0][:, :, 128 * m:128 * (m + 1)]
                        lhs1 = PTs[1][:, :, 128 * m:128 * (m + 1)]
                        st = stpool.tile([128, 2048], bf16, name="st", tag="st")
                        for c2 in range(4):
                            ch = 4 * chg + c2
                            nsl = slice(512 * ch, 512 * (ch + 1))
                            pc = pcpool.tile([128, 512], f32, name="pc", tag="pc")
                            h0 = slice(512 * ch, 512 * ch + 256)
                            h1 = slice(512 * ch + 256, 512 * (ch + 1))
                            # k order 0,1,1,0 shares the middle ldweights
                            nc.tensor.matmul(
                                pc[:, 0:256], lhs0, q8p[0][:, :, h0],
                                start=True, stop=False, perf_mode=DR,
                            )
                            nc.tensor.matmul(
                                pc[:, 0:256], lhs1, q8p[1][:, :, h0],
                                start=False, stop=True, perf_mode=DR,
                            )
                            nc.tensor.matmul(
                                pc[:, 256:512], lhs1, q8p[1][:, :, h1],
                                start=True, stop=False, perf_mode=DR,
                            )
                            nc.tensor.matmul(
                                pc[:, 256:512], lhs0, q8p[0][:, :, h1],
                                start=False, stop=True, perf_mode=DR,
                            )
                            ssl = slice(512 * c2, 512 * (c2 + 1))
                            if bmod and ch % bmod == bmod - 1:
                                pcs = stpool.tile(
                                    [128, 512], bf16, name="pcs", tag="pcs",
                                    bufs=3,
                                )
                                nc.scalar.activation(
                                    pcs, pc, ACT.Copy, scale=alph[m]
                                )
                                nc.vector.tensor_tensor(
                                    st[:, ssl], pcs, qb[m][:, nsl], op=ALU.add
                                )
                            else:
                                nc.vector.scalar_tensor_tensor(
                                    st[:, ssl], pc, alph[m], qb[m][:, nsl],
                                    op0=ALU.mult, op1=ALU.add,
                                )
                        out_eng.dma_start(
                            out_d[s, 128 * m:128 * (m + 1),
                                  2048 * chg:2048 * (chg + 1)],
                            st,
                        )

                    return step

                return [make_step(m, chg) for m in range(CT) for chg in range(2)]

            # ---- schedule ----
            loads = {0: emit_load(0)}
            params = emit_params()
            if BS > 1:
                loads[1] = emit_load(1)

            def round_robin(main, side):
                """Emit main steps with side steps threaded between them."""
                per = (len(side) + len(main) - 1) // max(len(main), 1)
                k = 0
                for f in main:
                    f()
                    for _ in range(per):
                        if k < len(side):
                            side[k]()
                            k += 1
                while k < len(side):
                    side[k]()
                    k += 1

            q8p0, qb0 = loads[0]
            ctx0, stepsA0, stepsB0 = mm1_steps(0, q8p0, qb0)
            for w in mm1_flat(ctx0, stepsA0, 0):
                w()

            # s0's softmax/passB/SE/PT phase; s1's pass A threads through it
            s0_rest = [lambda: softmax_pair(0, ctx0, 0)]
            s0_rest.extend(stepsB0)
            s0_rest.append(lambda: softmax_pair(0, ctx0, 2))
            m2_0_holder = {}

            def s0_sept():
                alph0, PTs0 = se_pt(0, ctx0, params)
                m2_0_holder["m2"] = mm2_steps(0, q8p0, qb0, alph0, PTs0)

            s0_rest.append(s0_sept)

            if BS > 1:
                q8p1, qb1 = loads[1]
                ctx1, stepsA1, stepsB1 = mm1_steps(1, q8p1, qb1)

                s1_passA = mm1_flat(ctx1, stepsA1, 1)
                s1_passA.append(lambda: softmax_pair(1, ctx1, 0))

                # phase 2: s0's softmax/passB/SE/PT runs ALONE — s1's input
                # chunks are still in flight, and PE is in-order: a stalled
                # s1 transpose would block s0's ready matmuls behind it
                for f0 in s0_rest:
                    f0()
                # phase 3: s0 MM2/store vs s1's whole MM1 pipeline (s1 data
                # is resident by now)
                round_robin(m2_0_holder["m2"], s1_passA + stepsB1)
                softmax_pair(1, ctx1, 2)
                alph1, PTs1 = se_pt(1, ctx1, params)
                for mstep in mm2_steps(1, q8p1, qb1, alph1, PTs1):
                    mstep()
            else:
                for f0 in s0_rest:
                    f0()
                for mstep in m2_0_holder["m2"]:
                    mstep()

    nc.compile()
    _BUILT = nc
    return nc


def kernel(**inputs):
    global LAST_RESULTS
    from concourse.bass_utils import run_bass_kernel_spmd

    x = np.ascontiguousarray(np.asarray(inputs["x"], dtype=np.float32))
    gamma = np.asarray(inputs["gamma"], dtype=np.float32)
    W1 = np.ascontiguousarray(np.asarray(inputs["W1"], dtype=np.float32))
    b1 = np.asarray(inputs["b1"], dtype=np.float32)
    W2 = np.ascontiguousarray(np.asarray(inputs["W2"], dtype=np.float32))
    b2 = np.asarray(inputs["b2"], dtype=np.float32)

    nc = _build()

    xr = x.reshape(B, C, HW)
    b1c = np.ascontiguousarray(b1.reshape(R, 1))
    b2c = np.ascontiguousarray(b2.reshape(C, 1))
    gc = np.ascontiguousarray(gamma.reshape(1, 1))

    in_maps = []
    for c in range(NCORES):
        shard = np.ascontiguousarray(xr[BS * c: BS * (c + 1)])
        in_maps.append(
            {"x": shard, "w1": W1, "b1": b1c, "w2": W2, "b2": b2c, "gam": gc}
        )

    res = run_bass_kernel_spmd(
        nc, in_maps, core_ids=list(range(NCORES)), trace=TRACE
    )
    LAST_RESULTS = res

    out = np.concatenate(
        [np.asarray(r["out"]).astype(np.float32) for r in res.results], axis=0
    )
    return out.reshape(B, C, H, W)
